# revision 12
# baseline (speedup 1.0000x reference)
"""Agent self-attention Trainium2 Bass kernel.

B=4, N=8192, DIM=1024, H=16, DH=64, M=128. 8 NeuronCores.
Sharding: core c owns batch c//2, sequence half c%2 (4096 tokens).
The ak softmax normalizer S (per batch/head/agent) and the agent values
ag are all-reduced across the pair of cores sharing a batch.

Per-core pipeline (all bf16 matmuls, fp32 accumulation):
  ph1: x -> xT (PE transpose); qkv projection; gates; ak_sim -> exp ->
       partial row sums S_c, E spilled to DRAM; qa_sim -> softmax (local)
       -> spilled to DRAM; v spilled to DRAM.
  ph2: AllReduce S over the batch pair; build talking-heads+1/S mixing
       weights.
  ph3: stream E back in head-grouped layout, mix via block-diag matmul,
       ag_partial = akM @ v; AllReduce ag.
  ph4: stream qa_attn back grouped, mix, out2 = qaM @ ag, apply sigmoid
       gate, transpose, output projection.
"""

import numpy as np

B, N, DIM, H, DH, M = 4, 8192, 1024, 16, 64, 128
SCALE = DH ** -0.5
NCORES = 8
NT = N // 2  # tokens per core


def agent_attn_body(tc, outs, ins, nt, groups):
    """Builds the per-core kernel body.

    ins: dict of APs: x (fp32 [nt,1024]), w_qk (bf16 [1024,2048]),
      w_v (bf16 [1024,1024]), a_t (bf16 [16,64,128]),
      wblk_ak/wblk_qa (bf16 [128,128]), w_gate (bf16 [1024,16]),
      b_gate (bf16 [1,16]), w_out (bf16 [1024,1024])
    outs: dict with out (fp32 [nt,1024])
    """
    import concourse.bass as bass
    import concourse.mybir as mybir
    from concourse.masks import make_identity
    from contextlib import ExitStack

    nc = tc.nc
    fp32 = mybir.dt.float32
    bf16 = mybir.dt.bfloat16
    Exp = mybir.ActivationFunctionType.Exp
    Copy = mybir.ActivationFunctionType.Copy
    Sigmoid = mybir.ActivationFunctionType.Sigmoid
    AX = mybir.AxisListType.X

    x_in = ins["x"]
    w_qk = ins["w_qk"]
    w_v = ins["w_v"]
    a_t = ins["a_t"]
    wblk_ak = ins["wblk_ak"]
    wblk_qa = ins["wblk_qa"]
    w_gate = ins["w_gate"]
    b_gate = ins["b_gate"]
    w_out = ins["w_out"]
    out_d = outs["out"]

    NCH = nt // 512   # 512-token chunks
    NIC = nt // 128   # 128-token chunks

    ctx = ExitStack()
    with ctx:
        # ---------------- constant / resident pools ----------------
        const = ctx.enter_context(tc.tile_pool(name="const", bufs=1))
        res = ctx.enter_context(tc.tile_pool(name="res", bufs=1))
        dram = ctx.enter_context(tc.tile_pool(name="dram", bufs=1, space="DRAM"))

        ident = const.tile([128, 128], bf16)
        make_identity(nc, ident)
        ones1 = const.tile([1, 128], bf16)
        nc.gpsimd.memset(ones1, 1.0)

        w_qk_sb = const.tile([128, 8, 2048], bf16)
        nc.sync.dma_start(out=w_qk_sb, in_=w_qk.rearrange("(k p) f -> p k f", p=128))
        w_v_sb = const.tile([128, 8, 1024], bf16)
        nc.sync.dma_start(out=w_v_sb, in_=w_v.rearrange("(k p) f -> p k f", p=128))
        w_out_sb = const.tile([128, 8, 1024], bf16)
        nc.sync.dma_start(out=w_out_sb, in_=w_out.rearrange("(t p) f -> p t f", p=128))
        w_gate_sb = const.tile([128, 8, 16], bf16)
        nc.sync.dma_start(out=w_gate_sb, in_=w_gate.rearrange("(k p) f -> p k f", p=128))
        bgate_sb = const.tile([1, 16], bf16)
        nc.sync.dma_start(out=bgate_sb, in_=b_gate)
        # a_t [h, d, m] -> sbuf [(hh d)=128, hpair=8, m=128]
        a_sb = const.tile([128, 8, 128], bf16)
        nc.sync.dma_start(out=a_sb, in_=a_t.rearrange("(hf hh) d m -> (hh d) hf m", hh=2))
        wblk_ak_sb = const.tile([128, 128], bf16)
        nc.sync.dma_start(out=wblk_ak_sb, in_=wblk_ak)
        wblk_qa_sb = const.tile([128, 128], bf16)
        nc.sync.dma_start(out=wblk_qa_sb, in_=wblk_qa)

        gates_sb = res.tile([128, NIC, 16], fp32)
        s_acc = res.tile([128, 16, NCH], fp32)
        dg = res.tile([128, 16], fp32)
        wblk_mo = res.tile([128, 16, 128], bf16)
        ag_sb = res.tile([128, 16, 64], fp32)
        agb = res.tile([128, 16, 64], bf16)

        # DRAM scratch
        espill = dram.tile([16, 128, nt], bf16)       # [h, m, j]
        qaspill = dram.tile([16, nt, 128], bf16)      # [h, i, m]
        vspill = dram.tile([nt, 1024], bf16)          # [j, (x d)]
        s_ccin = dram.tile([128, 16], fp32)
        s_ccout = dram.tile([128, 16], fp32)
        ag_ccin = dram.tile([128, 1024], fp32)
        ag_ccout = dram.tile([128, 1024], fp32)

        # PSUM pools
        ps_big = ctx.enter_context(tc.tile_pool(name="ps_big", bufs=3, space="PSUM"))
        ps_sm = ctx.enter_context(tc.tile_pool(name="ps_sm", bufs=4, space="PSUM"))
        nc.gpsimd.memset(ag_sb, 0.0)

        # ---------------- phase 1 ----------------
        with tc.tile_pool(name="ph1", bufs=3) as ph1, \
             tc.tile_pool(name="ph1xt", bufs=2) as ph1xt:
            for c in range(NCH):
                xt = ph1xt.tile([128, 8, 512], bf16, tag="xt")
                for s in range(4):
                    ic = c * 4 + s
                    xl = ph1.tile([128, 1024], fp32, tag="xl")
                    nc.sync.dma_start(out=xl, in_=x_in[ic * 128:(ic + 1) * 128, :])
                    xb = ph1.tile([128, 1024], bf16, tag="xb")
                    nc.vector.tensor_copy(out=xb, in_=xl)
                    for dt_ in range(8):
                        tp = ps_sm.tile([128, 128], bf16, tag="sm")
                        nc.tensor.transpose(tp, xb[:, dt_ * 128:(dt_ + 1) * 128], ident)
                        nc.scalar.activation(xt[:, dt_, s * 128:(s + 1) * 128], tp, Copy)
                # gates
                for s in range(4):
                    ic = c * 4 + s
                    gp = ps_sm.tile([128, 16], fp32, tag="sm")
                    for kc in range(8):
                        nc.tensor.matmul(gp, lhsT=xt[:, kc, s * 128:(s + 1) * 128],
                                         rhs=w_gate_sb[:, kc, :],
                                         start=(kc == 0), stop=False)
                    nc.tensor.matmul(gp, lhsT=ones1, rhs=bgate_sb, start=False, stop=True)
                    nc.scalar.activation(gates_sb[:, ic, :], gp, Sigmoid)
                # k features + ak_sim + E spill
                for ft in range(8):
                    kp = ps_big.tile([128, 512], fp32, tag="big")
                    for kc in range(8):
                        nc.tensor.matmul(kp, lhsT=w_qk_sb[:, kc, 1024 + ft * 128:1024 + (ft + 1) * 128],
                                         rhs=xt[:, kc, :], start=(kc == 0), stop=(kc == 7))
                    kt = ph1.tile([128, 512], bf16, tag="kt")
                    nc.vector.tensor_copy(out=kt, in_=kp)
                    for hh in range(2):
                        h = ft * 2 + hh
                        ap_ = ps_big.tile([128, 512], fp32, tag="big")
                        nc.tensor.matmul(ap_, lhsT=a_sb[hh * 64:(hh + 1) * 64, ft, :],
                                         rhs=kt[hh * 64:(hh + 1) * 64, :],
                                         start=True, stop=True)
                        et = ph1.tile([128, 512], bf16, tag="et")
                        nc.scalar.activation(et, ap_, Exp, accum_out=s_acc[:, h, c:c + 1])
                        nc.sync.dma_start(out=espill[h, :, c * 512:(c + 1) * 512], in_=et)
                # q features + qa_sim + local softmax + spill
                for ft in range(8):
                    qp = ps_big.tile([128, 512], fp32, tag="big")
                    for kc in range(8):
                        nc.tensor.matmul(qp, lhsT=w_qk_sb[:, kc, ft * 128:(ft + 1) * 128],
                                         rhs=xt[:, kc, :], start=(kc == 0), stop=(kc == 7))
                    qt = ph1.tile([128, 512], bf16, tag="kt")
                    nc.vector.tensor_copy(out=qt, in_=qp)
                    for hh in range(2):
                        h = ft * 2 + hh
                        for s in range(4):
                            ic = c * 4 + s
                            qap = ps_sm.tile([128, 128], fp32, tag="sm")
                            nc.tensor.matmul(qap, lhsT=qt[hh * 64:(hh + 1) * 64, s * 128:(s + 1) * 128],
                                             rhs=a_sb[hh * 64:(hh + 1) * 64, ft, :],
                                             start=True, stop=True)
                            eq = ph1.tile([128, 128], fp32, tag="eq")
                            sq = ph1.tile([128, 1], fp32, tag="sq")
                            nc.scalar.activation(eq, qap, Exp, accum_out=sq)
                            rq = ph1.tile([128, 1], fp32, tag="rq")
                            nc.vector.reciprocal(rq, sq)
                            qat = ph1.tile([128, 128], bf16, tag="qat")
                            nc.vector.tensor_scalar_mul(out=qat, in0=eq, scalar1=rq)
                            nc.sync.dma_start(out=qaspill[h, ic * 128:(ic + 1) * 128, :], in_=qat)
                # v (token-major) + spill
                for s in range(4):
                    ic = c * 4 + s
                    for nt_ in range(2):
                        vp = ps_big.tile([128, 512], fp32, tag="big")
                        for kc in range(8):
                            nc.tensor.matmul(vp, lhsT=xt[:, kc, s * 128:(s + 1) * 128],
                                             rhs=w_v_sb[:, kc, nt_ * 512:(nt_ + 1) * 512],
                                             start=(kc == 0), stop=(kc == 7))
                        vt_ = ph1.tile([128, 512], bf16, tag="vt")
                        nc.vector.tensor_copy(out=vt_, in_=vp)
                        nc.sync.dma_start(out=vspill[ic * 128:(ic + 1) * 128, nt_ * 512:(nt_ + 1) * 512], in_=vt_)

        # ---------------- phase 2: S all-reduce ----------------
        s_red = res.tile([128, 16], fp32)
        nc.vector.reduce_sum(s_red, s_acc, axis=AX)
        nc.sync.dma_start(out=s_ccin, in_=s_red)
        nc.gpsimd.collective_compute(
            "AllReduce", mybir.AluOpType.add, replica_groups=groups,
            ins=[s_ccin.opt()], outs=[s_ccout.opt()])
        sg = res.tile([128, 16], fp32)
        for h in range(16):
            nc.sync.dma_start(
                out=sg[h * 8:(h + 1) * 8, :],
                in_=s_ccout.rearrange("(mo m8) h -> m8 mo h", m8=8)[:, :, h])
        nc.vector.reciprocal(dg, sg)
        for mo in range(16):
            nc.vector.tensor_scalar_mul(out=wblk_mo[:, mo, :], in0=wblk_ak_sb,
                                        scalar1=dg[:, mo:mo + 1])

        # ---------------- phase 3: mix + ag ----------------
        with tc.tile_pool(name="ph3eg", bufs=2) as ph3eg, \
             tc.tile_pool(name="ph3", bufs=2) as ph3:
            for c in range(NCH):
                eg = ph3eg.tile([128, 16, 512], bf16, tag="eg")
                for h in range(16):
                    nc.sync.dma_start(
                        out=eg[h * 8:(h + 1) * 8],
                        in_=espill[h].rearrange("(mo m8) j -> m8 mo j", m8=8)[:, :, c * 512:(c + 1) * 512])
                for sub in range(4):
                    jc = c * 4 + sub
                    akm = ph3.tile([128, 16, 128], bf16, tag="akm")
                    for mo in range(16):
                        mp = ps_sm.tile([128, 128], fp32, tag="sm")
                        nc.tensor.matmul(mp, lhsT=eg[:, mo, sub * 128:(sub + 1) * 128],
                                         rhs=wblk_mo[:, mo, :], start=True, stop=True)
                        nc.vector.tensor_copy(
                            out=akm[:, :, mo * 8:(mo + 1) * 8],
                            in_=mp.rearrange("j (x m8) -> j x m8", x=16))
                    vt_ = ph3.tile([128, 1024], bf16, tag="v3")
                    nc.sync.dma_start(out=vt_, in_=vspill[jc * 128:(jc + 1) * 128, :])
                    for x in range(16):
                        agp = ps_sm.tile([128, 64], fp32, tag="sm")
                        nc.tensor.matmul(agp, lhsT=akm[:, x, :],
                                         rhs=vt_[:, x * 64:(x + 1) * 64],
                                         start=True, stop=True)
                        nc.vector.tensor_add(out=ag_sb[:, x, :], in0=ag_sb[:, x, :], in1=agp)
        nc.sync.dma_start(out=ag_ccin, in_=ag_sb.rearrange("p x d -> p (x d)"))
        nc.gpsimd.collective_compute(
            "AllReduce", mybir.AluOpType.add, replica_groups=groups,
            ins=[ag_ccin.opt()], outs=[ag_ccout.opt()])
        agf = res.tile([128, 1024], fp32)
        nc.sync.dma_start(out=agf, in_=ag_ccout)
        nc.vector.tensor_copy(out=agb.rearrange("p x d -> p (x d)"), in_=agf)

        # ---------------- phase 4: qa mix + out2 + gate + out proj ----------------
        with tc.tile_pool(name="ph4", bufs=2) as ph4:
            for ic in range(NIC):
                eqg = ph4.tile([128, 16, 128], bf16, tag="eqg")
                for h in range(16):
                    nc.sync.dma_start(
                        out=eqg[h * 8:(h + 1) * 8],
                        in_=qaspill[h, ic * 128:(ic + 1) * 128, :].rearrange("(io i8) m -> i8 io m", i8=8))
                qam = ph4.tile([128, 16, 128], bf16, tag="qam")
                for io in range(16):
                    mp = ps_sm.tile([128, 128], fp32, tag="sm")
                    nc.tensor.matmul(mp, lhsT=eqg[:, io, :], rhs=wblk_qa_sb,
                                     start=True, stop=True)
                    nc.vector.tensor_copy(
                        out=qam[:, :, io * 8:(io + 1) * 8],
                        in_=mp.rearrange("m (x i8) -> m x i8", x=16))
                o = ph4.tile([128, 1024], bf16, tag="o")
                for x in range(16):
                    op2 = ps_sm.tile([128, 64], fp32, tag="sm")
                    nc.tensor.matmul(op2, lhsT=qam[:, x, :], rhs=agb[:, x, :],
                                     start=True, stop=True)
                    nc.vector.tensor_scalar_mul(out=o[:, x * 64:(x + 1) * 64], in0=op2,
                                                scalar1=gates_sb[:, ic, x:x + 1])
                ot = ph4.tile([128, 8, 128], bf16, tag="ot")
                for t in range(8):
                    tp = ps_sm.tile([128, 128], bf16, tag="sm")
                    nc.tensor.transpose(tp, o[:, t * 128:(t + 1) * 128], ident)
                    nc.scalar.activation(ot[:, t, :], tp, Copy)
                osb = ph4.tile([128, 1024], fp32, tag="osb")
                for nt_ in range(2):
                    pp = ps_big.tile([128, 512], fp32, tag="big")
                    for t in range(8):
                        nc.tensor.matmul(pp, lhsT=ot[:, t, :],
                                         rhs=w_out_sb[:, t, nt_ * 512:(nt_ + 1) * 512],
                                         start=(t == 0), stop=(t == 7))
                    nc.scalar.activation(osb[:, nt_ * 512:(nt_ + 1) * 512], pp, Copy)
                nc.sync.dma_start(out=out_d[ic * 128:(ic + 1) * 128, :], in_=osb)


def host_prep(inputs, nt=NT, ncores=NCORES):
    """Builds per-core in_maps (numpy) from full inputs."""
    import ml_dtypes
    bf = ml_dtypes.bfloat16
    x = np.ascontiguousarray(np.asarray(inputs["x"], np.float32))
    W_qkv = np.asarray(inputs["W_qkv"], np.float32)
    agent = np.asarray(inputs["agent_tokens"], np.float32)
    W_qa = np.asarray(inputs["W_qa"], np.float32)
    W_ak = np.asarray(inputs["W_ak"], np.float32)
    W_gate = np.asarray(inputs["W_gate"], np.float32)
    b_gate = np.asarray(inputs["b_gate"], np.float32)
    W_out = np.asarray(inputs["W_out"], np.float32)

    w_qk = np.ascontiguousarray(W_qkv[:, :2 * H * DH]).astype(bf)
    w_v = np.ascontiguousarray(W_qkv[:, 2 * H * DH:]).astype(bf)
    a_t = np.ascontiguousarray((agent * SCALE).transpose(0, 2, 1)).astype(bf)  # [h,d,m]
    wblk_ak = np.zeros((128, 128), np.float32)
    wblk_qa = np.zeros((128, 128), np.float32)
    hh = np.arange(16)
    for g in range(8):
        rows = hh[:, None] * 8 + g
        cols = hh[None, :] * 8 + g
        wblk_ak[rows, cols] = W_ak.T
        wblk_qa[rows, cols] = W_qa.T
    wblk_ak = wblk_ak.astype(bf)
    wblk_qa = wblk_qa.astype(bf)
    w_gate_b = W_gate.astype(bf)
    b_gate_b = b_gate.reshape(1, H).astype(bf)
    w_out_b = W_out.astype(bf)

    xs = x.reshape(-1, nt, DIM)  # [ncores, nt, DIM] (batch-major halves)
    assert xs.shape[0] == ncores
    in_maps = []
    for c in range(ncores):
        in_maps.append({
            "x": xs[c],
            "w_qk": w_qk, "w_v": w_v, "a_t": a_t,
            "wblk_ak": wblk_ak, "wblk_qa": wblk_qa,
            "w_gate": w_gate_b, "b_gate": b_gate_b, "w_out": w_out_b,
        })
    return in_maps


_BUILT = {}
LAST_EXEC_NS = None


def _build_full():
    if "nc" in _BUILT:
        return _BUILT["nc"]
    import concourse.bass as bass
    import concourse.mybir as mybir
    import concourse.tile as tile
    from concourse import bacc

    fp32 = mybir.dt.float32
    bf16 = mybir.dt.bfloat16
    nc = bacc.Bacc("TRN2", target_bir_lowering=False, debug=False,
                   num_devices=NCORES)
    ins = {
        "x": nc.dram_tensor("x", [NT, DIM], fp32, kind="ExternalInput").ap(),
        "w_qk": nc.dram_tensor("w_qk", [DIM, 2 * H * DH], bf16, kind="ExternalInput").ap(),
        "w_v": nc.dram_tensor("w_v", [DIM, H * DH], bf16, kind="ExternalInput").ap(),
        "a_t": nc.dram_tensor("a_t", [H, DH, M], bf16, kind="ExternalInput").ap(),
        "wblk_ak": nc.dram_tensor("wblk_ak", [128, 128], bf16, kind="ExternalInput").ap(),
        "wblk_qa": nc.dram_tensor("wblk_qa", [128, 128], bf16, kind="ExternalInput").ap(),
        "w_gate": nc.dram_tensor("w_gate", [DIM, H], bf16, kind="ExternalInput").ap(),
        "b_gate": nc.dram_tensor("b_gate", [1, H], bf16, kind="ExternalInput").ap(),
        "w_out": nc.dram_tensor("w_out", [H * DH, DIM], bf16, kind="ExternalInput").ap(),
    }
    outs = {"out": nc.dram_tensor("out", [NT, DIM], fp32, kind="ExternalOutput").ap()}
    groups = [[0, 1], [2, 3], [4, 5], [6, 7]]
    with tile.TileContext(nc) as tc:
        agent_attn_body(tc, outs, ins, NT, groups)
    nc.finalize()
    _BUILT["nc"] = nc
    return nc


def kernel(**inputs):
    global LAST_EXEC_NS
    import os
    from concourse import bass_utils
    nc = _build_full()
    in_maps = host_prep(inputs)
    trace = bool(os.environ.get("AGENT_TRACE"))
    res = bass_utils.run_bass_kernel_spmd(
        nc, in_maps, core_ids=list(range(NCORES)), trace=trace)
    if res.exec_time_ns is not None:
        LAST_EXEC_NS = res.exec_time_ns
    outs = [r["out"] for r in res.results]
    out = np.stack(outs).reshape(B, N, DIM).astype(np.float32)
    return out
